# revision 1
# baseline (speedup 1.0000x reference)
"""EncNet vq_codebook kernel for 8 Trainium2 NeuronCores (v2).

Math (per reference):
  xs = x[:, :, 0, :].T                         # (b, s, c)
  d2[s,k]   = x2[s] - 2*cross[s,k] + cw2[k]
  a         = softmax_k(sm[k] * d2)
  e[b,k,c]  = sum_s a*xs - (sum_s a)*cw[k,c]
  BN over (b,c) (training stats), relu, mean over k, fc, sigmoid
  out = x * scale[b,c]

Distribution: data-parallel over batch (2 batches per core); BN batch
stats all-reduced across the 8 cores as a (64,2) tensor.  A dummy
warmup AllReduce early in phase 1 absorbs the ncfw cold-start so the
critical-path AllReduce runs warm (~13-20us instead of 24-95us).

I/O is bf16/fp16 end to end: the host casts x to bf16 and precomputes
xsq = x^2 in fp16 (removing the on-chip square pass), and the kernel
writes a bf16 output that the host upcasts.

On-core layout: s-chunks of 128 land on PSUM partitions.  Per group of
1024 s-columns (8 subchunks of 128):
  - seed matmul: onesw^T @ smcw2 -> L bank = sm_k*cw2_k (512 cols)
  - per subchunk i, with x-chunk (c=128, s=128) bf16 as PE weights:
      rhs = ident          -> xt chunk (s, c)  (transpose for free)
      rhs = -2*sm_k*cw^T   -> accumulate -2*sm_k*cross into L
    and with xsq = x^2 (fp16) as weights:
      rhs = sm (fp16 tile) -> accumulate sm_k*x2[s] into L
  so L[s,k] = sm_k*d2[s,k] exactly (logits <= ~0.006; exp is safe
  without max subtraction).  A single fp16 x^2 matmul suffices: the
  k-constant part of any logit error cancels in the softmax and the
  per-k uniform factor cancels in BN (e_bn is invariant to per-k
  scaling of e).
  - exp (scalar, PSUM->SBUF bf16), zw row-sums + reciprocal (vector)
  - xtn[s, i, 0:128] = xt * rz (vector, PSUM evac), xtn[s, i, 128] = rz
  - e-agg: one 129-col matmul per subchunk accumulates [e | asum] into
    a (64, 129) PSUM bank across the whole batch.
"""

import sys

import numpy as np

try:
    import concourse.bass as bass  # noqa: F401
except ImportError:
    sys.path.insert(0, "/opt/trn_rl_repo")

import concourse.bacc as bacc
import concourse.bass as bass
import concourse.mybir as mybir
import concourse.tile as tile
from concourse.bass_utils import run_bass_kernel_spmd
from concourse._compat import get_trn_type
from ml_dtypes import bfloat16
float16 = np.float16

F32 = mybir.dt.float32
BF16 = mybir.dt.bfloat16
FP16 = mybir.dt.float16
ALU = mybir.AluOpType
ACTF = mybir.ActivationFunctionType
AX = mybir.AxisListType

N_CORES = 8
B, C, SEQ, K = 16, 128, 16384, 64
B_LOC = B // N_CORES           # 2 batches per core
BIG = 2048                     # DMA chunk (free dim)
GRP = 512                      # softmax group: 4 subchunks share PSUM banks
SUB = 128                      # s-subchunk = PSUM partition dim
N_SUB = GRP // SUB             # 8
BN_EPS = 1e-5


def build_program(seq=SEQ, b_loc=B_LOC, n_cores=N_CORES, big=BIG):
    n_big = seq // big
    n_grp = big // GRP

    nc = bacc.Bacc(
        get_trn_type() or "TRN2",
        target_bir_lowering=False,
        debug=False,
        num_devices=n_cores,
    )

    x_ap = nc.dram_tensor("x", [b_loc, C, seq], BF16, kind="ExternalInput").ap()
    out_ap = nc.dram_tensor("out", [b_loc, C, seq], BF16, kind="ExternalOutput").ap()

    def const_in(name, shape, dt):
        return nc.dram_tensor(name, shape, dt, kind="ExternalInput").ap()

    fold_d = const_in("fold_f32", [C, K], F32)
    ident_d = const_in("ident_bf", [C, C], BF16)
    cwt_sm_d = const_in("cwt_sm_bf", [C, K], BF16)
    smtile_d = const_in("smtile_fp16", [C, K], FP16)
    onesw_d = const_in("onesw_bf", [C, C], BF16)
    smcw2_d = const_in("smcw2_bf", [C, N_SUB * K], BF16)
    cw_rows_d = const_in("cw_rows", [K, C], F32)
    gamma_d = const_in("gamma_col", [K, 1], F32)
    beta_d = const_in("beta_col", [K, 1], F32)
    fc_wt_d = const_in("fc_wt_bf", [C, C], BF16)
    fc_b_d = const_in("fc_b_col", [C, 1], F32)
    invk_d = const_in("invk_col", [K, 1], BF16)

    with tile.TileContext(nc) as tc:
        with (
            tc.tile_pool(name="consts", bufs=1) as cpool,
            tc.tile_pool(name="xg", bufs=2) as xgp,
            tc.tile_pool(name="xsq", bufs=2) as xsqp,
            tc.tile_pool(name="soft", bufs=3) as softp,
            tc.tile_pool(name="cols", bufs=8) as colp,
            tc.tile_pool(name="xtn", bufs=3) as xtnp,
            tc.tile_pool(name="etail", bufs=4) as etailp,
            tc.tile_pool(name="eloc", bufs=2) as elocp,
            tc.tile_pool(name="scales", bufs=2) as scalep,
            tc.tile_pool(name="og", bufs=3) as ogp,
            tc.tile_pool(name="ps_xt", bufs=3, space="PSUM") as ps_xt,
            tc.tile_pool(name="ps_L", bufs=3, space="PSUM") as ps_L,
            tc.tile_pool(name="ps_e", bufs=1, space="PSUM") as ps_e,
            tc.tile_pool(name="dram", bufs=4, space="DRAM") as dram,
        ):
            # ---- load constants into SBUF once ----
            def load_const(dram_ap, shape, dt):
                t = cpool.tile(shape, dt, tag=dram_ap.tensor.name)
                nc.scalar.dma_start(out=t[:], in_=dram_ap[:])
                return t

            fold = load_const(fold_d, [C, K], F32)
            ident = load_const(ident_d, [C, C], BF16)
            cwt_sm = load_const(cwt_sm_d, [C, K], BF16)
            smtile = load_const(smtile_d, [C, K], FP16)
            onesw = load_const(onesw_d, [C, C], BF16)
            smcw2 = load_const(smcw2_d, [C, N_SUB * K], BF16)
            cw_rows = load_const(cw_rows_d, [K, C], F32)
            gamma = load_const(gamma_d, [K, 1], F32)
            beta = load_const(beta_d, [K, 1], F32)
            fc_wt = load_const(fc_wt_d, [C, C], BF16)
            fc_b = load_const(fc_b_d, [C, 1], F32)
            invk = load_const(invk_d, [K, 1], BF16)

            # ---- warmup collective: absorbs ncfw cold-start off the
            # critical path (runs on the cc stream during phase-1 compute)
            ccw_in = dram.tile([K, 1], F32, tag="ccw_in")
            ccw_out = dram.tile([K, 1], F32, tag="ccw_out")
            nc.sync.dma_start(out=ccw_in[:], in_=gamma[:])
            nc.gpsimd.collective_compute(
                "AllReduce",
                ALU.add,
                replica_groups=[list(range(n_cores))],
                ins=[ccw_in.opt()],
                outs=[ccw_out.opt()],
            )
            ccw_sb = etailp.tile([K, 1], F32, tag="ccw_sb")
            nc.sync.dma_start(out=ccw_sb[:], in_=ccw_out[:])

            # ---- phase 1: batch-interleaved aggregation [e | asum] ----
            # x stays resident in SBUF for the whole run (used again by the
            # phase-2 scale), so HBM traffic is one read + one write of x.
            # The two batches' groups are interleaved so the PE always has
            # independent matmul work while the scalar/vector softmax chain
            # of the previous group drains; each batch accumulates into its
            # own PSUM bank.  Each group's e-agg matmuls are additionally
            # deferred by one group (software pipelining).
            xres = []
            e_locs = []
            gsts = []
            e_pss = []
            e_firsts = [True] * b_loc
            for b in range(b_loc):
                e_ps_b = ps_e.tile([C, C + 1], F32, tag=f"e_ps{b}")
                e_pss.append(e_ps_b)
                xg = xgp.tile([C, seq], BF16, tag=f"xg{b}")
                xres.append(xg)

            def emit_eagg(bb, araw_p, xtn_p, last):
                # `last` = final group of batch bb -> close its accum chain
                for i in range(N_SUB):
                    nc.tensor.matmul(
                        e_pss[bb][0:K, :], lhsT=araw_p[:, i * K : (i + 1) * K],
                        rhs=xtn_p[:, i, 0 : C + 1],
                        start=e_firsts[bb], stop=(last and i == N_SUB - 1),
                        skip_group_check=True,
                    )
                    e_firsts[bb] = False

            pending = None  # (batch, araw, xtn) of the previous group
            for j in range(n_big):
                for b in range(b_loc):
                    jsl = slice(j * big, (j + 1) * big)
                    xg = xres[b]
                    xsq = xsqp.tile([C, big], FP16)
                    if j == 0:
                        # split the first chunk so compute starts sooner
                        q = big // 4
                        for qq in range(4):
                            ql = slice(qq * q, (qq + 1) * q)
                            nc.sync.dma_start(
                                out=xg[:, ql], in_=x_ap[b, :, ql])
                            nc.scalar.square(
                                xsq[:, ql], xg[:, jsl][:, ql])
                    else:
                        nc.sync.dma_start(out=xg[:, jsl], in_=x_ap[b, :, jsl])
                        nc.scalar.square(xsq[:], xg[:, jsl])
                    for g in range(n_grp):
                        g0 = j * big + g * GRP
                        xt_ps = ps_xt.tile([SUB, N_SUB * C], F32)
                        L_ps = ps_L.tile([SUB, N_SUB * K], F32)
                        # constant sm_k*cw2_k term seeds the whole L bank
                        nc.tensor.matmul(
                            L_ps[:], lhsT=onesw[:], rhs=smcw2[:],
                            start=True, stop=False, skip_group_check=True,
                        )
                        for i in range(N_SUB):
                            sl = slice(g0 + i * SUB, g0 + (i + 1) * SUB)
                            sql = slice(
                                g * GRP + i * SUB, g * GRP + (i + 1) * SUB)
                            # xt_ps spans 2 PSUM banks; re-mark the zero
                            # region at each bank boundary
                            nc.tensor.matmul(
                                xt_ps[:, i * C : (i + 1) * C],
                                lhsT=xg[:, sl], rhs=ident[:],
                                start=(i % 4 == 0), stop=(i == N_SUB - 1),
                                skip_group_check=True,
                            )
                            nc.tensor.matmul(
                                L_ps[:, i * K : (i + 1) * K],
                                lhsT=xg[:, sl], rhs=cwt_sm[:],
                                start=False, stop=False,
                                skip_group_check=True,
                            )
                            nc.tensor.matmul(
                                L_ps[:, i * K : (i + 1) * K],
                                lhsT=xsq[:, sql], rhs=smtile[:],
                                start=False, stop=(i == N_SUB - 1),
                                skip_group_check=True,
                            )
                        # araw = exp(sm_k*d2) directly (cw2 already in L)
                        araw = softp.tile([SUB, N_SUB * K], BF16, tag="araw")
                        nc.scalar.activation(araw[:], L_ps[:], ACTF.Exp)
                        zw = colp.tile([SUB, N_SUB], F32, tag="zw")
                        nc.vector.tensor_reduce(
                            zw[:],
                            araw[:].rearrange("p (g k) -> p g k", g=N_SUB),
                            AX.X, ALU.add,
                        )
                        # rz lands directly in the xtn z-columns (bf16);
                        # the TT below re-reads it via a broadcast view
                        xtn = xtnp.tile([SUB, N_SUB, C + 8], BF16)
                        with nc.allow_low_precision(
                            reason="rz in bf16: per-s scale rides both the "
                            "e numerator and the asum column identically"
                        ):
                            nc.vector.reciprocal(
                                xtn[:, :, C : C + 1],
                                zw[:].rearrange("p (g o) -> p g o", o=1),
                            )
                        nc.vector.tensor_tensor(
                            xtn[:, :, 0:C],
                            xt_ps[:].rearrange("p (g c) -> p g c", g=N_SUB),
                            xtn[:, :, C : C + 1].broadcast_to(
                                [SUB, N_SUB, C]),
                            ALU.mult,
                        )
                        if pending is not None:
                            emit_eagg(*pending)
                        pending = (
                            b, araw, xtn,
                            j == n_big - 1 and g == n_grp - 1,
                        )
            emit_eagg(*pending)

            for b in range(b_loc):
                # ---- per-batch local e + stats ----
                e_sb = etailp.tile([K, C + 1], F32, tag=f"e_sb{b}")
                nc.vector.tensor_copy(e_sb[:], e_pss[b][0:K, :])
                easm = etailp.tile([K, C], F32, tag="easm")
                nc.vector.tensor_scalar(
                    out=easm[:], in0=cw_rows[:], scalar1=e_sb[:, C : C + 1],
                    scalar2=None, op0=ALU.mult,
                )
                e_loc = elocp.tile([K, C], F32)
                nc.vector.tensor_tensor(
                    e_loc[:], e_sb[:, 0:C], easm[:], ALU.add)
                e_locs.append(e_loc)
                stats = etailp.tile([K, 2], F32, tag=f"stats{b}")
                nc.vector.tensor_reduce(stats[:, 0:1], e_loc[:], AX.X, ALU.add)
                esq = etailp.tile([K, C], F32, tag="esq")
                nc.vector.tensor_tensor(esq[:], e_loc[:], e_loc[:], ALU.mult)
                nc.vector.tensor_reduce(stats[:, 1:2], esq[:], AX.X, ALU.add)
                gsts.append(stats)

            # ---- all-reduce BN stats across cores ----
            stats = etailp.tile([K, 2], F32, tag="stats_sum")
            nc.vector.tensor_tensor(stats[:], gsts[0][:], gsts[1][:], ALU.add)
            cc_in = dram.tile([K, 2], F32, tag="cc_in")
            cc_out = dram.tile([K, 2], F32, tag="cc_out")
            nc.sync.dma_start(out=cc_in[:], in_=stats[:])
            nc.gpsimd.collective_compute(
                "AllReduce",
                ALU.add,
                replica_groups=[list(range(n_cores))],
                ins=[cc_in.opt()],
                outs=[cc_out.opt()],
            )
            gst = etailp.tile([K, 2], F32, tag="gst_sum")
            nc.sync.dma_start(out=gst[:], in_=cc_out[:])

            # ---- BN affine + relu + mean_k + fc + sigmoid (tiny) ----
            n_tot = float(B * C)  # stats population: all b, all c
            mean = colp.tile([K, 1], F32, tag="mean")
            nc.vector.tensor_scalar(
                out=mean[:], in0=gst[:, 0:1], scalar1=1.0 / n_tot, scalar2=None,
                op0=ALU.mult,
            )
            ex2 = colp.tile([K, 1], F32, tag="ex2")
            nc.vector.tensor_scalar(
                out=ex2[:], in0=gst[:, 1:2], scalar1=1.0 / n_tot, scalar2=None,
                op0=ALU.mult,
            )
            msq = colp.tile([K, 1], F32, tag="msq")
            nc.vector.tensor_tensor(msq[:], mean[:], mean[:], ALU.mult)
            varep = colp.tile([K, 1], F32, tag="varep")
            nc.vector.tensor_tensor(varep[:], ex2[:], msq[:], ALU.subtract)
            nc.vector.tensor_scalar(
                out=varep[:], in0=varep[:], scalar1=BN_EPS, scalar2=None, op0=ALU.add
            )
            stdv = colp.tile([K, 1], F32, tag="stdv")
            nc.scalar.sqrt(stdv[:], varep[:])
            rstd = colp.tile([K, 1], F32, tag="rstd")
            nc.vector.reciprocal(rstd[:], stdv[:])
            psc = colp.tile([K, 1], F32, tag="psc")
            nc.vector.tensor_tensor(psc[:], gamma[:], rstd[:], ALU.mult)
            mps = colp.tile([K, 1], F32, tag="mps")
            nc.vector.tensor_tensor(mps[:], mean[:], psc[:], ALU.mult)
            pofs = colp.tile([K, 1], F32, tag="pofs")
            nc.vector.tensor_tensor(pofs[:], beta[:], mps[:], ALU.subtract)

            scale_cols = []
            for b in range(b_loc):
                reb = etailp.tile([K, C], BF16, tag="reb")
                nc.scalar.activation(
                    reb[:], e_locs[b][:], ACTF.Relu, bias=pofs[:], scale=psc[:]
                )
                en_ps = ps_e.tile([C, C + 1], F32, tag="e_ps0")
                nc.tensor.matmul(
                    en_ps[:, 0:1], lhsT=reb[:], rhs=invk[:], start=True, stop=True
                )
                en_sb = colp.tile([C, 1], BF16, tag="en_sb")
                nc.vector.tensor_copy(en_sb[:], en_ps[:, 0:1])
                fc_ps = ps_e.tile([C, C + 1], F32, tag="e_ps1")
                nc.tensor.matmul(
                    fc_ps[:, 0:1], lhsT=fc_wt[:], rhs=en_sb[:], start=True, stop=True
                )
                sc = scalep.tile([C, 1], F32)
                nc.scalar.activation(sc[:], fc_ps[:, 0:1], ACTF.Sigmoid, bias=fc_b[:])
                scale_cols.append(sc)

            # ---- phase 2: out = x * scale (x still resident in SBUF) ----
            for b in range(b_loc):
                for j in range(n_big):
                    jsl = slice(j * big, (j + 1) * big)
                    og = ogp.tile([C, big], BF16)
                    nc.vector.tensor_scalar(
                        out=og[:], in0=xres[b][:, jsl],
                        scalar1=scale_cols[b][:], scalar2=None, op0=ALU.mult,
                    )
                    nc.sync.dma_start(out=out_ap[b, :, jsl], in_=og[:])

    nc.compile()
    return nc


def make_const_inputs(codewords, smoothing, bn_weight, bn_bias, fc_w, fc_b):
    cw = np.asarray(codewords, np.float32)        # (K, C)
    sm = np.asarray(smoothing, np.float32)        # (K,)
    cw2 = (cw * cw).sum(1)                        # (K,)
    consts = {
        "fold_f32": np.concatenate(
            [np.eye(K, dtype=np.float32), np.eye(K, dtype=np.float32)], axis=0
        ),
        "ident_bf": np.eye(C, dtype=bfloat16),
        "cwt_sm_bf": (cw.T * (-2.0 * sm)[None, :]).astype(bfloat16),  # (C,K)
        "smtile_fp16": np.tile(sm.astype(float16)[None, :], (C, 1)),
        "onesw_bf": np.full((C, C), 1.0 / C, dtype=bfloat16),
        "smcw2_bf": np.tile((sm * cw2)[None, :], (C, N_SUB)).astype(bfloat16),
        "cw_rows": np.ascontiguousarray(-cw),
        "gamma_col": np.asarray(bn_weight, np.float32).reshape(K, 1),
        "beta_col": np.asarray(bn_bias, np.float32).reshape(K, 1),
        "fc_wt_bf": np.ascontiguousarray(np.asarray(fc_w, np.float32).T).astype(
            bfloat16
        ),
        "fc_b_col": np.asarray(fc_b, np.float32).reshape(C, 1),
        "invk_col": np.full((K, 1), 1.0 / K, bfloat16),
    }
    return consts


_NC_CACHE = {}


def _get_program():
    key = (SEQ, B_LOC, N_CORES, BIG)
    if key not in _NC_CACHE:
        _NC_CACHE[key] = build_program(*key)
    return _NC_CACHE[key]


def _run(inputs, trace=False, trace_kwargs=None):
    x = np.asarray(inputs["x"], np.float32)
    assert x.shape == (B, C, 1, SEQ), x.shape
    xs = np.ascontiguousarray(x.reshape(B, C, SEQ)).astype(bfloat16)
    consts = make_const_inputs(
        inputs["codewords"], inputs["smoothing"], inputs["bn_weight"],
        inputs["bn_bias"], inputs["fc_w"], inputs["fc_b"],
    )
    in_maps = [
        {
            "x": np.ascontiguousarray(xs[i * B_LOC : (i + 1) * B_LOC]),
            **consts,
        }
        for i in range(N_CORES)
    ]
    nc = _get_program()
    res = run_bass_kernel_spmd(
        nc, in_maps, core_ids=list(range(N_CORES)), trace=trace,
        **(trace_kwargs or {}),
    )
    out = np.concatenate([res.results[i]["out"] for i in range(N_CORES)], axis=0)
    return out.astype(np.float32).reshape(B, C, 1, SEQ), res


def kernel(**inputs):
    out, _ = _run(inputs)
    return out

